# revision 9
# baseline (speedup 1.0000x reference)
"""Focal Gaussian loss (EDT heatmap + focal MSE) on 8 Trainium2 cores.

Data-parallel over batch: each core processes B/8 = 2 images end to end,
producing per-core partial sums (sum of focal_factor, sum of focal*mse).
The host combines the 8 cores' partials and applies the global
normalization:

    out = SCALE * mean(focal*mse) / (mean(focal) + 0.01)

Row EDT: distance-to-nearest-fg within the row via two DVE scans
(count-since-last-fg recurrence, forward + reverse). Column pass:
min-conv with the parabola t^2 over a +-T window, computed in bf16
(d2 integers <= 256 are exact in bf16, and any pixel the window or
rounding affects has heatmap < exp(-T^2/8) ~ 4e-6, so the final scalar
moves by < ~1e-5 relative). DVE does the bf16 2x mins; ACT and GpSimd
do the parabola adds; PE does the row<->column transposes.
"""

import numpy as np

B, H, W = 16, 512, 512
N_CORES = 8
IPC = B // N_CORES  # images per core
T = 10              # column min-conv window radius
BIG = 1.0e6
BIG2 = 1.0e12
SCALE = 2.0
EPS = 0.01
P = 128
RB = H // P  # row blocks
CB = W // P  # col blocks
WPAD = 512 + 2 * T
NFREE = IPC * RB * W  # free elems per partition in merged layout

_CACHE = {}


def build_program():
    import concourse.bacc as bacc
    import concourse.mybir as mybir
    import concourse.tile as tile

    f32 = mybir.dt.float32
    bf16 = mybir.dt.bfloat16
    Alu = mybir.AluOpType
    Act = mybir.ActivationFunctionType

    nc = bacc.Bacc(
        "TRN2", target_bir_lowering=False, debug=False, num_devices=N_CORES
    )

    inp_d = nc.dram_tensor("inputs", [IPC, H, W], f32, kind="ExternalInput").ap()
    tgt_d = nc.dram_tensor("targets", [IPC, H, W], f32, kind="ExternalInput").ap()
    identb_d = nc.dram_tensor("identb", [P, P], bf16, kind="ExternalInput").ap()
    tsq_d = nc.dram_tensor("tsq", [P, T + 1], f32, kind="ExternalInput").ap()
    part_d = nc.dram_tensor("partials", [P, 2], f32, kind="ExternalOutput").ap()

    with tile.TileContext(nc) as tc:
        with (
            tc.tile_pool(name="const", bufs=1) as cpool,
            tc.tile_pool(name="io", bufs=1) as iopool,
            tc.tile_pool(name="work", bufs=1) as wpool,
            tc.tile_pool(name="tmp", bufs=3) as tpool,
            tc.tile_pool(name="psum", bufs=4, space="PSUM") as ppool,
        ):
            identb = cpool.tile([P, P], bf16)
            nc.sync.dma_start(identb[:], identb_d[:])
            tsq = cpool.tile([P, T + 1], f32)
            nc.sync.dma_start(tsq[:], tsq_d[:])
            ones = cpool.tile([P, W], f32)
            nc.vector.memset(ones[:], 1.0)
            bias015 = cpool.tile([P, 1], f32)
            nc.vector.memset(bias015[:], 0.15)
            partials = cpool.tile([P, 2], f32)

            tgt = iopool.tile([P, IPC, RB, W], f32, tag="tgt")
            nc.sync.dma_start(
                tgt[:], tgt_d.rearrange("i (a p) w -> p i a w", p=P)
            )
            inp = iopool.tile([P, IPC, RB, W], f32, tag="inp")
            nc.sync.dma_start(
                inp[:], inp_d.rearrange("i (a p) w -> p i a w", p=P)
            )

            # --- row EDT: dl/dr = count since last fg (fwd/rev scans) ---
            invt = wpool.tile([P, IPC, RB, W], f32, tag="invt")
            nc.gpsimd.tensor_scalar(
                invt[:], tgt[:], -1.0, 1.0, Alu.mult, Alu.add
            )
            dl = wpool.tile([P, IPC, RB, W], f32, tag="dl")
            dr = wpool.tile([P, IPC, RB, W], f32, tag="dr")
            for i in range(IPC):
                for rb in range(RB):
                    nc.vector.tensor_tensor_scan(
                        dl[:, i, rb],
                        ones[:],
                        invt[:, i, rb],
                        BIG,
                        Alu.add,
                        Alu.mult,
                    )
                    nc.vector.tensor_tensor_scan(
                        dr[:, i, rb, ::-1],
                        ones[:],
                        invt[:, i, rb, ::-1],
                        BIG,
                        Alu.add,
                        Alu.mult,
                    )
            # f = min(dl, dr) -> dr; f2 (bf16) via ACT square
            nc.vector.tensor_tensor(dr[:], dl[:], dr[:], op=Alu.min)
            f2r = wpool.tile([P, IPC, RB, W], bf16, tag="f2r")
            nc.scalar.square(f2r[:], dr[:])

            # --- transpose to column-major with +-T row padding (bf16) ---
            f2T = wpool.tile([P, IPC, CB, WPAD], bf16, tag="f2T")
            nc.vector.memset(f2T[:, :, :, 0:T], BIG2)
            nc.vector.memset(f2T[:, :, :, T + 512 : WPAD], BIG2)
            for i in range(IPC):
                for cb in range(CB):
                    ps = ppool.tile([P, 512], bf16, tag="psT")
                    for rb in range(RB):
                        nc.tensor.transpose(
                            ps[:, rb * P : (rb + 1) * P],
                            f2r[:, i, rb, cb * P : (cb + 1) * P],
                            identb[:],
                        )
                    nc.scalar.copy(f2T[:, i, cb, T : T + 512], ps[:])

            # --- column min-conv: d2[i] = min_r (r^2 + min(f2[i-r], f2[i+r]))
            # DVE: bf16 2x mins; ACT/GpSimd: the r^2 adds
            C0 = T
            acc = wpool.tile([P, IPC, CB, 512], bf16, tag="acc")

            def sl(off):
                return f2T[:, :, :, C0 + off : C0 + off + 512]

            for r in range(1, T + 1):
                pm = tpool.tile([P, IPC, CB, 512], bf16, tag="pm")
                nc.vector.tensor_tensor(pm[:], sl(-r), sl(r), op=Alu.min)
                tm = tpool.tile([P, IPC, CB, 512], bf16, tag="tm")
                if r % 2 == 0:
                    nc.scalar.activation(
                        tm[:], pm[:], Act.Identity, bias=tsq[:, r : r + 1]
                    )
                else:
                    nc.gpsimd.tensor_scalar_add(tm[:], pm[:], float(r * r))
                if r == 1:
                    nc.vector.tensor_tensor(acc[:], sl(0), tm[:], op=Alu.min)
                else:
                    nc.vector.tensor_tensor(acc[:], acc[:], tm[:], op=Alu.min)

            # --- transpose back + heat = exp(-d2/8) (f32, reuses inp tile
            # after sigmoid below consumes it... order: sigmoid first)
            pred = wpool.tile([P, IPC, RB, W], f32, tag="pred")
            nc.scalar.activation(pred[:], inp[:], Act.Sigmoid)
            heat = inp  # reuse
            for i in range(IPC):
                for rb in range(RB):
                    ph = ppool.tile([P, 512], bf16, tag="psH")
                    for cb in range(CB):
                        nc.tensor.transpose(
                            ph[:, cb * P : (cb + 1) * P],
                            acc[:, i, cb, rb * P : (rb + 1) * P],
                            identb[:],
                        )
                    nc.scalar.activation(
                        heat[:, i, rb], ph[:], Act.Exp, scale=-0.125
                    )

            # --- focal MSE (tgt is exactly 0/1):
            # 1-pt = pred + pos - 2*pos*pred, alpha_t = 0.7*pos + 0.15
            m_ = invt  # reuse
            nc.vector.tensor_mul(m_[:], tgt[:], pred[:])
            q_ = dl  # reuse
            nc.vector.scalar_tensor_tensor(
                q_[:], m_[:], -2.0, pred[:], Alu.mult, Alu.add
            )
            nc.gpsimd.tensor_tensor(q_[:], q_[:], tgt[:], op=Alu.add)
            q2 = m_  # reuse
            nc.scalar.square(q2[:], q_[:])
            alpha = dr  # reuse
            nc.scalar.activation(
                alpha[:], tgt[:], Act.Identity, bias=bias015[:], scale=0.7
            )
            nc.vector.scalar_tensor_tensor(
                alpha[:], alpha[:], 1.0, q2[:], Alu.mult, Alu.mult,
                accum_out=partials[:, 0:1],
            )
            nc.gpsimd.tensor_tensor(pred[:], pred[:], heat[:], op=Alu.subtract)
            nc.gpsimd.tensor_mul(pred[:], pred[:], pred[:])
            nc.vector.scalar_tensor_tensor(
                q_[:], alpha[:], 1.0, pred[:], Alu.mult, Alu.mult,
                accum_out=partials[:, 1:2],
            )

            nc.sync.dma_start(part_d[:], partials[:])

    nc.compile()
    return nc


def host_constants():
    import ml_dtypes

    identb = np.eye(P, dtype=ml_dtypes.bfloat16)
    tsq = np.broadcast_to(
        (np.arange(T + 1, dtype=np.float32) ** 2), (P, T + 1)
    ).copy()
    return identb, tsq


def make_in_maps(inputs, targets):
    identb, tsq = host_constants()
    in_maps = []
    for c in range(N_CORES):
        sl_ = slice(c * IPC, (c + 1) * IPC)
        in_maps.append(
            {
                "inputs": np.ascontiguousarray(inputs[sl_, 0]),
                "targets": np.ascontiguousarray(targets[sl_, 0]),
                "identb": identb,
                "tsq": tsq,
            }
        )
    return in_maps


def combine_partials(partial_list):
    """partial_list: one [128, 2] array per core -> final scalar."""
    sf = 0.0
    sl_ = 0.0
    for parts in partial_list:
        p64 = parts.astype(np.float64)
        sf += p64[:, 0].sum()
        sl_ += p64[:, 1].sum()
    n = float(B * H * W)
    out = SCALE * (sl_ / n) / (sf / n + EPS)
    return np.float32(out)


def kernel(inputs, targets):
    from concourse.bass_utils import run_bass_kernel_spmd

    if "nc" not in _CACHE:
        _CACHE["nc"] = build_program()
    nc = _CACHE["nc"]

    in_maps = make_in_maps(inputs, targets)
    res = run_bass_kernel_spmd(nc, in_maps, list(range(N_CORES)))
    return combine_partials([r["partials"] for r in res.results])


# revision 13
# speedup vs baseline: 3.2519x; 3.2519x over previous
"""Focal Gaussian loss (EDT heatmap + focal MSE) on 8 Trainium2 cores.

Data-parallel over batch: each core processes B/8 = 2 images end to end,
producing per-core partial sums (sum of focal_factor, sum of focal*mse).
The host combines the 8 cores' partials and applies the global
normalization:

    out = SCALE * mean(focal*mse) / (mean(focal) + 0.01)

Row EDT: distance-to-nearest-fg within the row via two DVE scans
(count-since-last-fg recurrence, forward + reverse). Column pass:
min-conv with the parabola t^2 over a +-T window, computed in bf16
(d2 integers <= 256 are exact in bf16, and any pixel the window or
rounding affects has heatmap < exp(-T^2/8) ~ 4e-6, so the final scalar
moves by < ~1e-5 relative). DVE does the bf16 2x mins; ACT and GpSimd
do the parabola adds; PE does the row<->column transposes.
"""

import numpy as np

B, H, W = 16, 512, 512
N_CORES = 8
IPC = B // N_CORES  # images per core
T = 10              # column min-conv window radius
BIG = 1.0e6
BIG2 = 1.0e12
SCALE = 2.0
EPS = 0.01
P = 128
RB = H // P  # row blocks
CB = W // P  # col blocks
WPAD = 512 + 2 * T
NFREE = IPC * RB * W  # free elems per partition in merged layout

_CACHE = {}


def build_program():
    import concourse.bacc as bacc
    import concourse.mybir as mybir
    import concourse.tile as tile

    f32 = mybir.dt.float32
    bf16 = mybir.dt.bfloat16
    Alu = mybir.AluOpType
    Act = mybir.ActivationFunctionType

    nc = bacc.Bacc(
        "TRN2", target_bir_lowering=False, debug=False, num_devices=N_CORES
    )

    inp_d = nc.dram_tensor("inputs", [IPC, H, W], f32, kind="ExternalInput").ap()
    tgt_d = nc.dram_tensor("targets", [IPC, H, W], f32, kind="ExternalInput").ap()
    identb_d = nc.dram_tensor("identb", [P, P], bf16, kind="ExternalInput").ap()
    tsq_d = nc.dram_tensor("tsq", [P, T + 1], f32, kind="ExternalInput").ap()
    part_d = nc.dram_tensor("partials", [P, 2], f32, kind="ExternalOutput").ap()

    with tile.TileContext(nc) as tc:
        with (
            tc.tile_pool(name="const", bufs=1) as cpool,
            tc.tile_pool(name="io", bufs=1) as iopool,
            tc.tile_pool(name="work", bufs=1) as wpool,
            tc.tile_pool(name="tmp", bufs=3) as tpool,
            tc.tile_pool(name="psum", bufs=4, space="PSUM") as ppool,
        ):
            identb = cpool.tile([P, P], bf16)
            nc.sync.dma_start(identb[:], identb_d[:])
            tsq = cpool.tile([P, T + 1], f32)
            nc.sync.dma_start(tsq[:], tsq_d[:])
            ones = cpool.tile([P, W], f32)
            nc.vector.memset(ones[:], 1.0)
            bias015 = cpool.tile([P, 1], f32)
            nc.vector.memset(bias015[:], 0.15)
            partials = cpool.tile([P, 2], f32)

            tgt = iopool.tile([P, IPC, RB, W], f32, tag="tgt")
            inp = iopool.tile([P, IPC, RB, W], f32, tag="inp")
            tgt_r = tgt_d.rearrange("i (a p) w -> p i a w", p=P)
            inp_r = inp_d.rearrange("i (a p) w -> p i a w", p=P)
            for i in range(IPC):
                nc.sync.dma_start(tgt[:, i], tgt_r[:, i])
                nc.sync.dma_start(inp[:, i], inp_r[:, i])

            # --- row EDT: dl/dr = count since last fg (fwd/rev scans) ---
            invt = wpool.tile([P, IPC, RB, W], f32, tag="invt")
            dl = wpool.tile([P, IPC, RB, W], f32, tag="dl")
            dr = wpool.tile([P, IPC, RB, W], f32, tag="dr")
            f2r = wpool.tile([P, IPC, RB, W], bf16, tag="f2r")
            # pred early: ACT is idle during the scans
            pred = wpool.tile([P, IPC, RB, W], f32, tag="pred")
            nc.scalar.activation(pred[:], inp[:], Act.Sigmoid)
            # w = 1 - 2*tgt (for 1-pt = pred*w + tgt), GpSimd f32
            wt = wpool.tile([P, IPC, RB, W], f32, tag="wt")
            for i in range(IPC):
                nc.gpsimd.tensor_scalar(
                    invt[:, i], tgt[:, i], -1.0, 1.0, Alu.mult, Alu.add
                )
            nc.gpsimd.tensor_scalar(
                wt[:], tgt[:], -2.0, 1.0, Alu.mult, Alu.add
            )
            for i in range(IPC):
                for rb in range(RB):
                    nc.vector.tensor_tensor_scan(
                        dl[:, i, rb],
                        ones[:],
                        invt[:, i, rb],
                        BIG,
                        Alu.add,
                        Alu.mult,
                    )
                    nc.vector.tensor_tensor_scan(
                        dr[:, i, rb, ::-1],
                        ones[:],
                        invt[:, i, rb, ::-1],
                        BIG,
                        Alu.add,
                        Alu.mult,
                    )
                # f = min(dl, dr) -> dr; f2 (bf16) via ACT square
                nc.vector.tensor_tensor(
                    dr[:, i], dl[:, i], dr[:, i], op=Alu.min
                )
                nc.scalar.square(f2r[:, i], dr[:, i])

            # --- transpose to column-major with +-T row padding (bf16) ---
            f2T = wpool.tile([P, IPC, CB, WPAD], bf16, tag="f2T")
            nc.vector.memset(f2T[:, :, :, 0:T], BIG2)
            nc.vector.memset(f2T[:, :, :, T + 512 : WPAD], BIG2)
            for i in range(IPC):
                for cb in range(CB):
                    ps = ppool.tile([P, 512], bf16, tag="psT")
                    for rb in range(RB):
                        nc.tensor.transpose(
                            ps[:, rb * P : (rb + 1) * P],
                            f2r[:, i, rb, cb * P : (cb + 1) * P],
                            identb[:],
                        )
                    nc.scalar.copy(f2T[:, i, cb, T : T + 512], ps[:])

            # --- column min-conv: d2[i] = min_r (r^2 + min(f2[i-r], f2[i+r]))
            # DVE: bf16 2x mins (two independent chains); ACT: the r^2 adds
            C0 = T
            acc0 = wpool.tile([P, IPC, CB, 512], bf16, tag="acc0")
            acc1 = wpool.tile([P, IPC, CB, 512], bf16, tag="acc1")

            def sl(off):
                return f2T[:, :, :, C0 + off : C0 + off + 512]

            for r in range(1, T + 1):
                pm = tpool.tile([P, IPC, CB, 512], bf16, tag="pm")
                nc.vector.tensor_tensor(pm[:], sl(-r), sl(r), op=Alu.min)
                tm = tpool.tile([P, IPC, CB, 512], bf16, tag="tm")
                nc.scalar.activation(
                    tm[:], pm[:], Act.Identity, bias=tsq[:, r : r + 1]
                )
                acc = acc0 if r % 2 == 0 else acc1
                if r == 1:
                    nc.vector.tensor_tensor(acc[:], sl(0), tm[:], op=Alu.min)
                elif r == 2:
                    nc.vector.tensor_copy(acc[:], tm[:])
                else:
                    nc.vector.tensor_tensor(acc[:], acc[:], tm[:], op=Alu.min)
            nc.vector.tensor_tensor(acc0[:], acc0[:], acc1[:], op=Alu.min)
            acc = acc0

            # --- focal front half on GpSimd while DVE runs the min-conv:
            # q = 1-pt = pred*w + tgt
            q_ = invt  # reuse
            nc.gpsimd.tensor_tensor(q_[:], pred[:], wt[:], op=Alu.mult)
            nc.gpsimd.tensor_tensor(q_[:], q_[:], tgt[:], op=Alu.add)

            # --- transpose back + heat = exp(-d2/8) (f32)
            heat = inp  # reuse
            for i in range(IPC):
                for rb in range(RB):
                    ph = ppool.tile([P, 512], bf16, tag="psH")
                    for cb in range(CB):
                        nc.tensor.transpose(
                            ph[:, cb * P : (cb + 1) * P],
                            acc[:, i, cb, rb * P : (rb + 1) * P],
                            identb[:],
                        )
                    nc.scalar.activation(
                        heat[:, i, rb], ph[:], Act.Exp, scale=-0.125
                    )

            # --- focal tail: alpha_t = 0.7*pos + 0.15, focal = alpha*q^2,
            # loss = focal * (pred-heat)^2; accum_out gives the two sums
            alpha = dr  # reuse
            nc.scalar.activation(
                alpha[:], tgt[:], Act.Identity, bias=bias015[:], scale=0.7
            )
            q2 = dl  # reuse
            nc.scalar.square(q2[:], q_[:])
            nc.vector.scalar_tensor_tensor(
                alpha[:], alpha[:], 1.0, q2[:], Alu.mult, Alu.mult,
                accum_out=partials[:, 0:1],
            )
            for i in range(IPC):
                for rb in range(RB):
                    nc.vector.tensor_tensor(
                        pred[:, i, rb], pred[:, i, rb], heat[:, i, rb],
                        op=Alu.subtract,
                    )
                    nc.vector.tensor_mul(
                        pred[:, i, rb], pred[:, i, rb], pred[:, i, rb]
                    )
            nc.vector.scalar_tensor_tensor(
                q2[:], alpha[:], 1.0, pred[:], Alu.mult, Alu.mult,
                accum_out=partials[:, 1:2],
            )

            nc.sync.dma_start(part_d[:], partials[:])

    nc.compile()
    return nc


def host_constants():
    import ml_dtypes

    identb = np.eye(P, dtype=ml_dtypes.bfloat16)
    tsq = np.broadcast_to(
        (np.arange(T + 1, dtype=np.float32) ** 2), (P, T + 1)
    ).copy()
    return identb, tsq


def make_in_maps(inputs, targets):
    identb, tsq = host_constants()
    in_maps = []
    for c in range(N_CORES):
        sl_ = slice(c * IPC, (c + 1) * IPC)
        in_maps.append(
            {
                "inputs": np.ascontiguousarray(inputs[sl_, 0]),
                "targets": np.ascontiguousarray(targets[sl_, 0]),
                "identb": identb,
                "tsq": tsq,
            }
        )
    return in_maps


def combine_partials(partial_list):
    """partial_list: one [128, 2] array per core -> final scalar."""
    sf = 0.0
    sl_ = 0.0
    for parts in partial_list:
        p64 = parts.astype(np.float64)
        sf += p64[:, 0].sum()
        sl_ += p64[:, 1].sum()
    n = float(B * H * W)
    out = SCALE * (sl_ / n) / (sf / n + EPS)
    return np.float32(out)


def kernel(inputs, targets):
    from concourse.bass_utils import run_bass_kernel_spmd

    if "nc" not in _CACHE:
        _CACHE["nc"] = build_program()
    nc = _CACHE["nc"]

    in_maps = make_in_maps(inputs, targets)
    res = run_bass_kernel_spmd(nc, in_maps, list(range(N_CORES)))
    return combine_partials([r["partials"] for r in res.results])


# revision 15
# speedup vs baseline: 3.3298x; 1.0240x over previous
"""Focal Gaussian loss (EDT heatmap + focal MSE) on 8 Trainium2 cores.

Data-parallel over batch: each core processes B/8 = 2 images end to end,
producing per-core partial sums (sum of focal_factor, sum of focal*mse).
The host combines the 8 cores' partials and applies the global
normalization:

    out = SCALE * mean(focal*mse) / (mean(focal) + 0.01)

Row EDT: count-since-last-fg recurrence via DVE scans (forward +
reverse), with a sentinel column between row-chunks so one scan
instruction covers a whole image. Column pass: min-conv with the
parabola t^2 over a +-T window in bf16 (d2 integers <= 256 are exact in
bf16; any pixel the window or rounding affects has heatmap <
exp(-T^2/8) = e^-8, measured scalar impact ~1e-6 relative). DVE does
bf16 2x mins, ACT the parabola adds, PE the row<->column transposes,
GpSimd the f32 elementwise focal prep.
"""

import numpy as np

B, H, W = 16, 512, 512
N_CORES = 8
IPC = B // N_CORES  # images per core
T = 8               # column min-conv window radius
W1 = W + 1          # row width + sentinel column
SENT = 64.0         # sentinel invt value: forces count >= 64 at row starts
BIG = 1.0e6
BIG2 = 1.0e12
SCALE = 2.0
EPS = 0.01
P = 128
RB = H // P  # row blocks
CB = W // P  # col blocks
WPAD = 512 + 2 * T

_CACHE = {}


def build_program():
    import concourse.bacc as bacc
    import concourse.mybir as mybir
    import concourse.tile as tile

    f32 = mybir.dt.float32
    bf16 = mybir.dt.bfloat16
    Alu = mybir.AluOpType
    Act = mybir.ActivationFunctionType

    nc = bacc.Bacc(
        "TRN2", target_bir_lowering=False, debug=False, num_devices=N_CORES
    )

    inp_d = nc.dram_tensor("inputs", [IPC, H, W], f32, kind="ExternalInput").ap()
    tgt_d = nc.dram_tensor("targets", [IPC, H, W], f32, kind="ExternalInput").ap()
    identb_d = nc.dram_tensor("identb", [P, P], bf16, kind="ExternalInput").ap()
    tsq_d = nc.dram_tensor("tsq", [P, T + 1], f32, kind="ExternalInput").ap()
    part_d = nc.dram_tensor("partials", [P, 2], f32, kind="ExternalOutput").ap()

    with tile.TileContext(nc) as tc:
        with (
            tc.tile_pool(name="const", bufs=1) as cpool,
            tc.tile_pool(name="io", bufs=1) as iopool,
            tc.tile_pool(name="work", bufs=1) as wpool,
            tc.tile_pool(name="tmp", bufs=3) as tpool,
            tc.tile_pool(name="psum", bufs=4, space="PSUM") as ppool,
        ):
            identb = cpool.tile([P, P], bf16)
            nc.sync.dma_start(identb[:], identb_d[:])
            tsq = cpool.tile([P, T + 1], f32)
            nc.sync.dma_start(tsq[:], tsq_d[:])
            ones = cpool.tile([P, RB * W1], f32)
            nc.vector.memset(ones[:], 1.0)
            bias015 = cpool.tile([P, 1], f32)
            nc.vector.memset(bias015[:], 0.15)
            partials = cpool.tile([P, 2], f32)

            tgt = iopool.tile([P, IPC, RB, W], f32, tag="tgt")
            inp = iopool.tile([P, IPC, RB, W], f32, tag="inp")
            tgt_r = tgt_d.rearrange("i (a p) w -> p i a w", p=P)
            inp_r = inp_d.rearrange("i (a p) w -> p i a w", p=P)
            for i in range(IPC):
                nc.sync.dma_start(tgt[:, i], tgt_r[:, i])
            for i in range(IPC):
                nc.sync.dma_start(inp[:, i], inp_r[:, i])

            # --- row EDT: dl/dr = count since last fg (fwd/rev scans),
            # sentinel column resets the count between row-chunks
            invt = wpool.tile([P, IPC, RB, W1], f32, tag="invt")
            dl = wpool.tile([P, IPC, RB, W1], f32, tag="dl")
            dr = wpool.tile([P, IPC, RB, W1], f32, tag="dr")
            nc.vector.memset(invt[:, :, :, W:W1], SENT)
            for i in range(IPC):
                nc.gpsimd.tensor_scalar(
                    invt[:, i, :, 0:W], tgt[:, i], -1.0, 1.0, Alu.mult, Alu.add
                )
            # pred early: ACT is idle during the scans
            pred = wpool.tile([P, IPC, RB, W], f32, tag="pred")
            nc.scalar.activation(pred[:], inp[:], Act.Sigmoid)

            sql = wpool.tile([P, IPC, RB, W], bf16, tag="sql")
            sqr = wpool.tile([P, IPC, RB, W], bf16, tag="sqr")
            for i in range(IPC):
                iflat = invt[:, i].rearrange("p a w -> p (a w)")
                lflat = dl[:, i].rearrange("p a w -> p (a w)")
                rflat = dr[:, i].rearrange("p a w -> p (a w)")
                nc.vector.tensor_tensor_scan(
                    lflat, ones[:], iflat, BIG, Alu.add, Alu.mult
                )
                nc.vector.tensor_tensor_scan(
                    rflat[:, ::-1], ones[:], iflat[:, ::-1], BIG,
                    Alu.add, Alu.mult,
                )
                nc.scalar.square(sql[:, i], dl[:, i, :, 0:W])
                nc.scalar.square(sqr[:, i], dr[:, i, :, 0:W])
            # f2 = min(dl^2, dr^2) in bf16 (2x), in place into sql
            f2r = sql
            nc.vector.tensor_tensor(f2r[:], sql[:], sqr[:], op=Alu.min)

            # --- transpose to column-major with +-T row padding (bf16) ---
            f2T = wpool.tile([P, IPC, CB, WPAD], bf16, tag="f2T")
            nc.vector.memset(f2T[:, :, :, 0:T], BIG2)
            nc.vector.memset(f2T[:, :, :, T + 512 : WPAD], BIG2)
            for i in range(IPC):
                for cb in range(CB):
                    ps = ppool.tile([P, 512], bf16, tag="psT")
                    for rb in range(RB):
                        nc.tensor.transpose(
                            ps[:, rb * P : (rb + 1) * P],
                            f2r[:, i, rb, cb * P : (cb + 1) * P],
                            identb[:],
                        )
                    nc.scalar.copy(f2T[:, i, cb, T : T + 512], ps[:])

            # --- column min-conv: d2[i] = min_r (r^2 + min(f2[i-r], f2[i+r]))
            # DVE: bf16 2x mins (two independent chains); ACT: the r^2 adds
            C0 = T
            acc0 = wpool.tile([P, IPC, CB, 512], bf16, tag="sqr")
            acc1 = wpool.tile([P, IPC, CB, 512], bf16, tag="acc1")

            def sl(off):
                return f2T[:, :, :, C0 + off : C0 + off + 512]

            for r in range(1, T + 1):
                pm = tpool.tile([P, IPC, CB, 512], bf16, tag="pm")
                nc.vector.tensor_tensor(pm[:], sl(-r), sl(r), op=Alu.min)
                tm = tpool.tile([P, IPC, CB, 512], bf16, tag="pm")
                nc.scalar.activation(
                    tm[:], pm[:], Act.Identity, bias=tsq[:, r : r + 1]
                )
                acc = acc0 if r % 2 == 0 else acc1
                if r == 1:
                    nc.vector.tensor_tensor(acc[:], sl(0), tm[:], op=Alu.min)
                elif r == 2:
                    nc.vector.tensor_copy(acc[:], tm[:])
                else:
                    nc.vector.tensor_tensor(acc[:], acc[:], tm[:], op=Alu.min)
            nc.vector.tensor_tensor(acc0[:], acc0[:], acc1[:], op=Alu.min)
            acc = acc0

            # --- focal front half on GpSimd while DVE runs the min-conv:
            # q = 1-pt = pred*w + tgt, w = 1-2*tgt (dl slot is free now)
            wt = dl
            nc.gpsimd.tensor_scalar(
                wt[:, :, :, 0:W], tgt[:], -2.0, 1.0, Alu.mult, Alu.add
            )
            q_ = invt  # reuse
            nc.gpsimd.tensor_tensor(
                q_[:, :, :, 0:W], pred[:], wt[:, :, :, 0:W], op=Alu.mult
            )
            nc.gpsimd.tensor_tensor(
                q_[:, :, :, 0:W], q_[:, :, :, 0:W], tgt[:], op=Alu.add
            )

            # --- focal tail: alpha_t = 0.7*pos + 0.15, focal = alpha*q^2,
            # loss = focal * (pred-heat)^2; accum_out gives the two sums
            alpha = dr  # reuse (padded layout, use [:, :, :, 0:W])
            nc.scalar.activation(
                alpha[:, :, :, 0:W], tgt[:], Act.Identity,
                bias=bias015[:], scale=0.7,
            )
            q2 = wt  # dl slot again (wt consumed by q_)
            nc.scalar.square(q2[:, :, :, 0:W], q_[:, :, :, 0:W])
            nc.vector.scalar_tensor_tensor(
                alpha[:, :, :, 0:W], alpha[:, :, :, 0:W], 1.0,
                q2[:, :, :, 0:W], Alu.mult, Alu.mult,
                accum_out=partials[:, 0:1],
            )
            # --- transpose back + heat = exp(-d2/8) (f32)
            heat = inp  # reuse
            for i in range(IPC):
                for rb in range(RB):
                    ph = ppool.tile([P, 512], bf16, tag="psH")
                    for cb in range(CB):
                        nc.tensor.transpose(
                            ph[:, cb * P : (cb + 1) * P],
                            acc[:, i, cb, rb * P : (rb + 1) * P],
                            identb[:],
                        )
                    nc.scalar.activation(
                        heat[:, i, rb], ph[:], Act.Exp, scale=-0.125
                    )

            # d = pred - heat on GpSimd per row-chunk (overlaps the exps),
            # then d^2 and the accumulating loss product on DVE
            for i in range(IPC):
                for rb in range(RB):
                    nc.gpsimd.tensor_tensor(
                        pred[:, i, rb], pred[:, i, rb], heat[:, i, rb],
                        op=Alu.subtract,
                    )
                nc.vector.tensor_mul(pred[:, i], pred[:, i], pred[:, i])
            nc.vector.scalar_tensor_tensor(
                q2[:, :, :, 0:W], alpha[:, :, :, 0:W], 1.0, pred[:],
                Alu.mult, Alu.mult,
                accum_out=partials[:, 1:2],
            )

            nc.sync.dma_start(part_d[:], partials[:])

    nc.compile()
    return nc


def host_constants():
    import ml_dtypes

    identb = np.eye(P, dtype=ml_dtypes.bfloat16)
    tsq = np.broadcast_to(
        (np.arange(T + 1, dtype=np.float32) ** 2), (P, T + 1)
    ).copy()
    return identb, tsq


def make_in_maps(inputs, targets):
    identb, tsq = host_constants()
    in_maps = []
    for c in range(N_CORES):
        sl_ = slice(c * IPC, (c + 1) * IPC)
        in_maps.append(
            {
                "inputs": np.ascontiguousarray(inputs[sl_, 0]),
                "targets": np.ascontiguousarray(targets[sl_, 0]),
                "identb": identb,
                "tsq": tsq,
            }
        )
    return in_maps


def combine_partials(partial_list):
    """partial_list: one [128, 2] array per core -> final scalar."""
    sf = 0.0
    sl_ = 0.0
    for parts in partial_list:
        p64 = parts.astype(np.float64)
        sf += p64[:, 0].sum()
        sl_ += p64[:, 1].sum()
    n = float(B * H * W)
    out = SCALE * (sl_ / n) / (sf / n + EPS)
    return np.float32(out)


def kernel(inputs, targets):
    from concourse.bass_utils import run_bass_kernel_spmd

    if "nc" not in _CACHE:
        _CACHE["nc"] = build_program()
    nc = _CACHE["nc"]

    in_maps = make_in_maps(inputs, targets)
    res = run_bass_kernel_spmd(nc, in_maps, list(range(N_CORES)))
    return combine_partials([r["partials"] for r in res.results])
